# revision 20
# baseline (speedup 1.0000x reference)
"""Trainium2 Bass kernel for ConfigurableMultiHeadAttention with
cum-thresholded (top-p style) softmax.

Sharding: data-parallel over (batch x query-row-half) -- core c handles
batch c//2, query rows (c%2)*512 .. +512, and ALL 16 heads for those
rows.  The head-mean of the attention matrix is core-local (no
collective, no host-side reduction); each core writes its own 512-row
slice of attn and of out = attn @ v.  Host just concatenates.

All matmuls run in fp16 (f32 PSUM accumulate): q/k/v and the weights
are cast to f16 on the host.

Cum-thresholded softmax without sort/cumsum: find the per-row cutoff
value c* where the ascending cumulative mass crosses 0.1*E, keep
e > lo, renormalize by the actual kept mass.  Root-finding is Illinois
false position (K=3), warm-started from a logE regression; probes are
fused compare-mult-accumulate ops on DVE plus Relu/Sign pairs on ACT.

Probe groups are HEAD-pair groups (heads 4j..4j+3 across all four
q-tiles): a group's scores only need two projection column blocks, so
its bisection starts while later projections still run, and different
groups' round chains interleave on the engines (no global round
barrier).  The kept mass S comes from tracking m at the accepted lower
bound during the rounds, so finalize masks need no accumulation -- the
per-head mask tiles stream straight into identity-diag matmuls on the
otherwise idle PE, which performs the 16-head mean in PSUM.
"""

import numpy as np

B, SQ, SKV, D, H, DH = 4, 1024, 1024, 1024, 16, 64
NCORES = 8
ROWS = 512          # q rows per core
NQT = ROWS // 128   # q-tiles per core (4)
NGRP = 4            # head-pair groups (heads 4j..4j+3)
NTP = 16            # probe tiles per group (4 qtiles x 4 heads)
K_ITERS = 3
CA, CB = 0.73950811, -5.77096104
LOM, HIM = 0.1418, 0.1865  # search window margins around predictor
TH, EPS, SCALE = 0.1, 1e-7, 0.125
LAM = 1.75                # model slope for initial endpoint masses
W_LO, W_HI = 0.04, 0.96   # false-position weight clip
N_ACT_PROBE = 6           # probe tiles per group on ACT (Relu+Sign pair)
N_ACT_PROBE_LAST = 7      # last group: ACT has no exp work to overlap
NHALF = 1                 # early rounds probe only the first 512 kv columns

_CACHE = {}


def _build_module():
    import concourse.bacc as bacc
    import concourse.mybir as mybir
    from concourse.tile import TileContext
    from concourse.bass import ds, ts
    from concourse.masks import make_identity

    f32, f16 = mybir.dt.float32, mybir.dt.float16
    AL = mybir.AluOpType
    AF = mybir.ActivationFunctionType

    nc = bacc.Bacc("TRN2", target_bir_lowering=False, debug=False,
                   enable_asserts=False, num_devices=NCORES)
    qTs = nc.dram_tensor("qTs", (D, ROWS), f16, kind="ExternalInput").ap()
    kT = nc.dram_tensor("kT", (D, SKV), f16, kind="ExternalInput").ap()
    vm = nc.dram_tensor("vm", (SKV, D), f16, kind="ExternalInput").ap()
    wqT = nc.dram_tensor("wqT", (D, D), f16, kind="ExternalInput").ap()
    wkT = nc.dram_tensor("wkT", (D, D), f16, kind="ExternalInput").ap()
    attn_o = nc.dram_tensor("attn_p", (ROWS, SKV), f16, kind="ExternalOutput").ap()
    out_o = nc.dram_tensor("out_p", (ROWS, D), f16, kind="ExternalOutput").ap()

    from contextlib import ExitStack
    with TileContext(nc) as tc:
        with ExitStack() as stk:
            # [128,1024] f16 slots shared by weight/k staging and e16
            big = stk.enter_context(tc.tile_pool(name="big", bufs=52))
            kqp = stk.enter_context(tc.tile_pool(name="kqp", bufs=8))
            vpool = stk.enter_context(tc.tile_pool(name="vp", bufs=1))
            maskp = stk.enter_context(tc.tile_pool(name="maskp", bufs=18))
            dgp = stk.enter_context(tc.tile_pool(name="dgp", bufs=6))
            attnp = stk.enter_context(tc.tile_pool(name="attn", bufs=2))
            atacc = stk.enter_context(tc.tile_pool(name="atacc", bufs=4))
            scrT = stk.enter_context(tc.tile_pool(name="scrT", bufs=2))
            scrA = stk.enter_context(tc.tile_pool(name="scrA", bufs=2))
            small = stk.enter_context(tc.tile_pool(name="small", bufs=5))
            aTp = stk.enter_context(tc.tile_pool(name="aTp", bufs=8))
            osbp = stk.enter_context(tc.tile_pool(name="osb", bufs=3))
            wcons = stk.enter_context(tc.tile_pool(name="wcons", bufs=1))
            pssc = stk.enter_context(tc.tile_pool(name="pssc", bufs=2, space="PSUM"))
            ps512 = stk.enter_context(tc.tile_pool(name="ps512", bufs=2, space="PSUM"))
            psat = stk.enter_context(tc.tile_pool(name="psat", bufs=2, space="PSUM"))

            bias_lo = wcons.tile([128, 1], f32, tag="blo")
            bias_hi = wcons.tile([128, 1], f32, tag="bhi")
            nc.vector.memset(bias_lo, CB - LOM)
            nc.vector.memset(bias_hi, CB + HIM)
            ident = wcons.tile([128, 128], f16, tag="ident")
            make_identity(nc, ident)

            # ---- stage weights / k / q / v (f16); v last (needed only at AV)
            wq, wk, kt = [], [], []
            for dc in range(8):
                t_ = big.tile([128, D], f16, tag="big", name="wk_sb")
                nc.sync.dma_start(t_, wkT[ts(dc, 128), :])
                wk.append(t_)
            for dc in range(8):
                t_ = big.tile([128, SKV], f16, tag="big", name="kt_sb")
                nc.sync.dma_start(t_, kT[ts(dc, 128), :])
                kt.append(t_)
            for dc in range(8):
                t_ = big.tile([128, D], f16, tag="big", name="wq_sb")
                nc.sync.dma_start(t_, wqT[ts(dc, 128), :])
                wq.append(t_)
            qt_sb = []
            for dc in range(8):
                t_ = kqp.tile([128, ROWS], f16, tag="qt", name="qt_sb")
                nc.sync.dma_start(t_, qTs[ts(dc, 128), :])
                qt_sb.append(t_)
            v_sb = vpool.tile([128, 8, D], f16, tag="v")
            for kc in range(8):
                nc.sync.dma_start(v_sb[:, kc, :], vm[ts(kc, 128), :])

            kp, qp = [], []
            e16s = {j: {} for j in range(NGRP)}   # group j -> {t: tile}
            st = {}                                # group state tiles
            at16acc = {}                           # qt -> f16 attn accumulator

            def emit_scores(j, g, E_t):
                # heads 2g, 2g+1 of group j (local h2=0,1);
                # tile index t = qt*4 + (g*2 + h2 - 4j)
                for qt in range(NQT):
                    for h2 in range(2):
                        t = qt * 4 + (g * 2 + h2 - 4 * j)
                        ps2 = pssc.tile([128, 1024], f32, tag="pssc")
                        for n in range(2):
                            nc.tensor.matmul(
                                out=ps2[:, ds(n * 512, 512)],
                                lhsT=qp[g][ds(h2 * 64, 64), ts(qt, 128)],
                                rhs=kp[g][ds(h2 * 64, 64), ds(n * 512, 512)],
                                start=True, stop=True,
                                tile_position=(h2 * 64, 0))
                        e16 = big.tile([128, SKV], f16, tag="big", name="e16")
                        nc.scalar.activation(e16, ps2, AF.Exp, scale=SCALE,
                                             accum_out=E_t[:, t:t + 1])
                        e16s[j][t] = e16

            def emit_rounds(j):
                s = st[j]
                E_t, lo, hi, thE = s["E"], s["lo"], s["hi"], s["thE"]
                T_t, N_t, m_t = s["T"], s["N"], s["m"]
                mlo, mhi, thEh = s["mlo"], s["mhi"], s["thEh"]
                lnE = small.tile([128, NTP], f32, tag="lnE")
                nc.scalar.activation(lnE, E_t, AF.Ln)
                nc.scalar.activation(lo, lnE, AF.Exp, scale=CA, bias=bias_lo)
                nc.scalar.activation(hi, lnE, AF.Exp, scale=CA, bias=bias_hi)
                nc.vector.tensor_scalar_mul(thE, E_t, TH)
                nc.vector.tensor_scalar_mul(mlo, thE, float(np.exp(-LAM * LOM)))
                nc.vector.tensor_scalar_mul(mhi, thE, float(np.exp(LAM * HIM)))
                nc.vector.tensor_scalar_mul(thEh, thE, 0.5)

                nact = N_ACT_PROBE if j < NGRP - 1 else N_ACT_PROBE_LAST
                ndve = NTP - nact
                for it in range(K_ITERS):
                    half = it < NHALF
                    wid = SKV // 2 if half else SKV
                    c_t = small.tile([128, NTP], f32, tag="c")
                    den = small.tile([128, NTP], f32, tag="den")
                    num = small.tile([128, NTP], f32, tag="num")
                    w_t = small.tile([128, NTP], f32, tag="w")
                    nc.vector.tensor_sub(den, mhi, mlo)
                    nc.vector.reciprocal(den, den)
                    nc.vector.tensor_sub(num, thE, mlo)
                    nc.vector.tensor_mul(w_t, num, den)
                    nc.vector.tensor_scalar(out=w_t, in0=w_t, scalar1=W_LO,
                                            scalar2=W_HI, op0=AL.max, op1=AL.min)
                    nc.vector.tensor_sub(c_t, hi, lo)
                    nc.vector.tensor_mul(c_t, c_t, w_t)
                    nc.vector.tensor_add(c_t, c_t, lo)
                    if half:
                        c2_t = small.tile([128, NTP], f32, tag="c2")
                        nc.vector.tensor_scalar_mul(c2_t, c_t, 2.0)
                    for t in range(NTP):
                        col = c_t[:, t:t + 1]
                        if t < ndve:
                            sT = scrT.tile([128, SKV], f16, tag="sT", name="sT")
                            nc.vector.scalar_tensor_tensor(
                                out=sT[:, ds(0, wid)], in0=e16s[j][t][:, ds(0, wid)],
                                scalar=col, in1=e16s[j][t][:, ds(0, wid)],
                                op0=AL.is_le, op1=AL.mult,
                                accum_out=m_t[:, t:t + 1])
                        else:
                            # full: R = sum relu(c-e); half: 2R via scale -2
                            sa = scrA.tile([128, SKV], f16, tag="sA", name="sa")
                            nc.scalar.activation(sa[:, ds(0, wid)],
                                                 e16s[j][t][:, ds(0, wid)], AF.Relu,
                                                 bias=(c2_t[:, t:t + 1] if half
                                                       else col),
                                                 scale=(-2.0 if half else -1.0),
                                                 accum_out=T_t[:, t:t + 1])
                            sg = scrA.tile([128, SKV], f16, tag="sA", name="sg")
                            nc.scalar.activation(sg[:, ds(0, wid)],
                                                 e16s[j][t][:, ds(0, wid)], AF.Sign,
                                                 bias=col, scale=-1.0,
                                                 accum_out=N_t[:, t:t + 1])
                    acols = ds(ndve, nact)
                    dcols = ds(0, ndve)
                    if half:
                        # m_half doubled to full-scale estimate
                        nc.vector.tensor_scalar_mul(m_t[:, dcols], m_t[:, dcols], 2.0)
                        # ACT: 2R in T_t, G' over 512; m = c*(G'+512) - 2R
                        nc.vector.tensor_scalar(
                            out=m_t[:, acols], in0=N_t[:, acols], scalar1=1.0,
                            scalar2=float(SKV // 2), op0=AL.mult, op1=AL.add)
                    else:
                        # ACT: R in T_t, G' over 1024; m = c*(G'+1024)/2 - R
                        nc.vector.tensor_scalar(
                            out=m_t[:, acols], in0=N_t[:, acols], scalar1=0.5,
                            scalar2=float(SKV // 2), op0=AL.mult, op1=AL.add)
                    nc.vector.tensor_mul(m_t[:, acols], m_t[:, acols], c_t[:, acols])
                    nc.vector.tensor_sub(m_t[:, acols], m_t[:, acols], T_t[:, acols])
                    # halved endpoint masses
                    hlo = small.tile([128, NTP], f32, tag="hlo")
                    hhi = small.tile([128, NTP], f32, tag="hhi")
                    nc.vector.scalar_tensor_tensor(
                        out=hlo, in0=mlo, scalar=0.5, in1=thEh,
                        op0=AL.mult, op1=AL.add)
                    nc.vector.scalar_tensor_tensor(
                        out=hhi, in0=mhi, scalar=0.5, in1=thEh,
                        op0=AL.mult, op1=AL.add)
                    # branch update
                    sel = small.tile([128, NTP], mybir.dt.uint8, tag="sel")
                    nc.vector.tensor_tensor(out=sel, in0=m_t, in1=thE, op=AL.is_lt)
                    nc.vector.copy_predicated(lo, sel, c_t)
                    nc.vector.copy_predicated(mlo, sel, m_t)
                    nc.vector.copy_predicated(mhi, sel, hhi)
                    nc.vector.tensor_tensor(out=sel, in0=m_t, in1=thE, op=AL.is_ge)
                    nc.vector.copy_predicated(hi, sel, c_t)
                    nc.vector.copy_predicated(mhi, sel, m_t)
                    nc.vector.copy_predicated(mlo, sel, hlo)

            def emit_av(qt):
                at16 = at16acc[qt]
                nc.sync.dma_start(attn_o[ts(qt, 128), :], at16)
                aTs = []
                for kc in range(8):
                    pst = ps512.tile([128, 128], f16, tag="ps512", name="pst")
                    nc.tensor.transpose(pst, at16[:, ts(kc, 128)], ident)
                    aT = aTp.tile([128, 128], f16, tag="aT")
                    nc.vector.tensor_copy(aT, pst)
                    aTs.append(aT)
                for half in range(2):
                    po = ps512.tile([128, 512], f32, tag="ps512")
                    for kc in range(8):
                        nc.tensor.matmul(
                            out=po, lhsT=aTs[kc],
                            rhs=v_sb[:, kc, ds(half * 512, 512)],
                            start=(kc == 0), stop=(kc == 7))
                    osb = osbp.tile([128, 512], f16, tag="osb")
                    nc.scalar.copy(osb, po)
                    nc.sync.dma_start(out_o[ts(qt, 128), ds(half * 512, 512)], osb)

            def emit_partial(j):
                # masks (with S accumulation) for the whole group, then r2,
                # then diag-scale + PE-accumulate and fold into the f16
                # accumulator (pool does the adds)
                s = st[j]
                lo, E_t, S_t, r2_t = s["lo"], s["E"], s["S"], s["r2"]
                mks = {}
                for qt in range(NQT):
                    for hh in range(4):
                        col = qt * 4 + hh
                        mk = maskp.tile([128, SKV], f16, tag="mk", name="mk")
                        nc.vector.scalar_tensor_tensor(
                            out=mk, in0=e16s[j][col], scalar=lo[:, col:col + 1],
                            in1=e16s[j][col], op0=AL.is_gt, op1=AL.mult,
                            accum_out=S_t[:, col:col + 1])
                        mks[col] = mk
                nc.vector.scalar_tensor_tensor(
                    out=S_t, in0=E_t, scalar=EPS, in1=S_t,
                    op0=AL.mult, op1=AL.add)
                nc.vector.reciprocal(r2_t, S_t)
                for qt in range(NQT):
                    pa = [psat.tile([128, 512], f32, tag="psat", name="pa")
                          for _ in range(2)]
                    for hh in range(4):
                        col = qt * 4 + hh
                        mk = mks[col]
                        dg = dgp.tile([128, 128], f16, tag="dg", name="dg")
                        nc.vector.tensor_scalar(
                            out=dg, in0=ident, scalar1=r2_t[:, col:col + 1],
                            scalar2=None, op0=AL.mult)
                        for half in range(2):
                            nc.tensor.matmul(
                                out=pa[half], lhsT=dg,
                                rhs=mk[:, ds(half * 512, 512)],
                                start=(hh == 0), stop=(hh == 3))
                    if j == 0:
                        acc = atacc.tile([128, SKV], f16, tag="acc", name="acc")
                        at16acc[qt] = acc
                        for half in range(2):
                            nc.scalar.activation(acc[:, ds(half * 512, 512)],
                                                 pa[half], AF.Copy, scale=1.0 / H)
                    else:
                        tmp16 = attnp.tile([128, SKV], f16, tag="attn", name="tmp16")
                        for half in range(2):
                            nc.scalar.activation(tmp16[:, ds(half * 512, 512)],
                                                 pa[half], AF.Copy, scale=1.0 / H)
                        nc.gpsimd.tensor_tensor(out=at16acc[qt], in0=at16acc[qt],
                                                in1=tmp16, op=AL.add)
                    if j == NGRP - 1:
                        emit_av(qt)

            # ---- projections interleaved with scores/rounds of ready groups
            for g in range(8):
                kp_g = kqp.tile([128, SKV], f16, tag="kp", name="kp_g", bufs=3)
                for half in range(2):
                    ps = ps512.tile([128, 512], f32, tag="ps512")
                    for dc in range(8):
                        nc.tensor.matmul(out=ps, lhsT=wk[dc][:, ts(g, 128)],
                                         rhs=kt[dc][:, ds(half * 512, 512)],
                                         start=(dc == 0), stop=(dc == 7))
                    nc.vector.tensor_copy(kp_g[:, ds(half * 512, 512)], ps)
                kp.append(kp_g)
                qp_g = kqp.tile([128, ROWS], f16, tag="qp", name="qp_g", bufs=3)
                ps = ps512.tile([128, 512], f32, tag="ps512")
                for dc in range(8):
                    nc.tensor.matmul(out=ps, lhsT=wq[dc][:, ts(g, 128)],
                                     rhs=qt_sb[dc],
                                     start=(dc == 0), stop=(dc == 7))
                nc.vector.tensor_copy(qp_g, ps)
                qp.append(qp_g)
                if g % 2 == 1:
                    j = g // 2
                    st[j] = {nm: small.tile([128, NTP], f32, tag=nm, name=nm)
                             for nm in ("E", "lo", "hi", "thE", "T", "N", "m",
                                        "mlo", "mhi", "thEh", "S", "r2")}
                    emit_scores(j, g - 1, st[j]["E"])
                    emit_scores(j, g, st[j]["E"])
                    emit_rounds(j)
                    emit_partial(j)

    nc.compile()
    return nc


def _get_module():
    if "nc" not in _CACHE:
        _CACHE["nc"] = _build_module()
    return _CACHE["nc"]


def kernel(q, k, v, Wq, Wk, k_mask=None):
    from concourse.bass_utils import run_bass_kernel_spmd

    nc = _get_module()
    f16 = np.float16
    qT = np.ascontiguousarray(q.transpose(0, 2, 1)).astype(f16)   # (B, D, SQ)
    kTf = np.ascontiguousarray(k.transpose(0, 2, 1)).astype(f16)  # (B, D, SKV)
    v16 = np.ascontiguousarray(v).astype(f16)
    wqT = np.ascontiguousarray(Wq.T).astype(f16)
    wkT = np.ascontiguousarray(Wk.T).astype(f16)
    in_maps = []
    for c in range(NCORES):
        b, r = c // 2, c % 2
        in_maps.append({
            "qTs": np.ascontiguousarray(qT[b][:, r * ROWS:(r + 1) * ROWS]),
            "kT": kTf[b],
            "vm": v16[b],
            "wqT": wqT,
            "wkT": wkT,
        })
    res = run_bass_kernel_spmd(nc, in_maps, core_ids=list(range(NCORES)))
    _CACHE["last_res"] = res
    attn = np.empty((B, SQ, SKV), np.float32)
    out = np.empty((B, SQ, D), np.float32)
    for c in range(NCORES):
        b, r = c // 2, c % 2
        attn[b, r * ROWS:(r + 1) * ROWS, :] = res.results[c]["attn_p"]
        out[b, r * ROWS:(r + 1) * ROWS, :] = res.results[c]["out_p"]
    return out, attn


# revision 22
# speedup vs baseline: 1.0203x; 1.0203x over previous
"""Trainium2 Bass kernel for ConfigurableMultiHeadAttention with
cum-thresholded (top-p style) softmax.

Sharding: data-parallel over (batch x query-row-half) -- core c handles
batch c//2, query rows (c%2)*512 .. +512, and ALL 16 heads for those
rows.  The head-mean of the attention matrix is core-local (no
collective, no host-side reduction); each core writes its own 512-row
slice of attn and of out = attn @ v.  Host just concatenates.

All matmuls run in fp16 (f32 PSUM accumulate): q/k/v and the weights
are cast to f16 on the host.

Cum-thresholded softmax without sort/cumsum: find the per-row cutoff
value c* where the ascending cumulative mass crosses 0.1*E, keep
e > lo, renormalize by the actual kept mass.  Root-finding is Illinois
false position (K=3), warm-started from a logE regression; probes are
fused compare-mult-accumulate ops on DVE plus Relu/Sign pairs on ACT.

Probe groups are HEAD-pair groups (heads 4j..4j+3 across all four
q-tiles): a group's scores only need two projection column blocks, so
its bisection starts while later projections still run, and different
groups' round chains interleave on the engines (no global round
barrier).  The kept mass S comes from tracking m at the accepted lower
bound during the rounds, so finalize masks need no accumulation -- the
per-head mask tiles stream straight into identity-diag matmuls on the
otherwise idle PE, which performs the 16-head mean in PSUM.
"""

import numpy as np

B, SQ, SKV, D, H, DH = 4, 1024, 1024, 1024, 16, 64
NCORES = 8
ROWS = 512          # q rows per core
NQT = ROWS // 128   # q-tiles per core (4)
NGRP = 4            # head-pair groups (heads 4j..4j+3)
NTP = 16            # probe tiles per group (4 qtiles x 4 heads)
K_ITERS = 3
CA, CB = 0.73950811, -5.77096104
LOM, HIM = 0.1418, 0.1865  # search window margins around predictor
TH, EPS, SCALE = 0.1, 1e-7, 0.125
LAM = 1.75                # model slope for initial endpoint masses
W_LO, W_HI = 0.04, 0.96   # false-position weight clip
N_ACT_PROBE = 6           # probe tiles per group on ACT (Relu+Sign pair)
N_ACT_PROBE_LAST = 7      # last group: ACT has no exp work to overlap
NHALF = 1                 # early rounds probe only the first 512 kv columns

_CACHE = {}


def _build_module():
    import concourse.bacc as bacc
    import concourse.mybir as mybir
    from concourse.tile import TileContext
    from concourse.bass import ds, ts
    from concourse.masks import make_identity

    f32, f16 = mybir.dt.float32, mybir.dt.float16
    AL = mybir.AluOpType
    AF = mybir.ActivationFunctionType

    nc = bacc.Bacc("TRN2", target_bir_lowering=False, debug=False,
                   enable_asserts=False, num_devices=NCORES)
    qTs = nc.dram_tensor("qTs", (D, ROWS), f16, kind="ExternalInput").ap()
    kT = nc.dram_tensor("kT", (D, SKV), f16, kind="ExternalInput").ap()
    vm = nc.dram_tensor("vm", (SKV, D), f16, kind="ExternalInput").ap()
    wqT = nc.dram_tensor("wqT", (D, D), f16, kind="ExternalInput").ap()
    wkT = nc.dram_tensor("wkT", (D, D), f16, kind="ExternalInput").ap()
    attn_o = nc.dram_tensor("attn_p", (ROWS, SKV), f16, kind="ExternalOutput").ap()
    out_o = nc.dram_tensor("out_p", (ROWS, D), f16, kind="ExternalOutput").ap()

    from contextlib import ExitStack
    with TileContext(nc) as tc:
        with ExitStack() as stk:
            # [128,1024] f16 slots shared by weight/k staging and e16
            big = stk.enter_context(tc.tile_pool(name="big", bufs=52))
            kqp = stk.enter_context(tc.tile_pool(name="kqp", bufs=8))
            vpool = stk.enter_context(tc.tile_pool(name="vp", bufs=1))
            maskp = stk.enter_context(tc.tile_pool(name="maskp", bufs=18))
            dgp = stk.enter_context(tc.tile_pool(name="dgp", bufs=6))
            atacc = stk.enter_context(tc.tile_pool(name="atacc", bufs=6))
            scrT = stk.enter_context(tc.tile_pool(name="scrT", bufs=2))
            scrA = stk.enter_context(tc.tile_pool(name="scrA", bufs=2))
            small = stk.enter_context(tc.tile_pool(name="small", bufs=5))
            aTp = stk.enter_context(tc.tile_pool(name="aTp", bufs=8))
            osbp = stk.enter_context(tc.tile_pool(name="osb", bufs=3))
            wcons = stk.enter_context(tc.tile_pool(name="wcons", bufs=1))
            pssc = stk.enter_context(tc.tile_pool(name="pssc", bufs=2, space="PSUM"))
            ps512 = stk.enter_context(tc.tile_pool(name="ps512", bufs=2, space="PSUM"))
            psat = stk.enter_context(tc.tile_pool(name="psat", bufs=2, space="PSUM"))

            bias_lo = wcons.tile([128, 1], f32, tag="blo")
            bias_hi = wcons.tile([128, 1], f32, tag="bhi")
            nc.vector.memset(bias_lo, CB - LOM)
            nc.vector.memset(bias_hi, CB + HIM)
            ident = wcons.tile([128, 128], f16, tag="ident")
            make_identity(nc, ident)
            hident = wcons.tile([128, 128], f16, tag="hident")
            nc.vector.tensor_scalar(out=hident, in0=ident, scalar1=float(H),
                                    scalar2=None, op0=AL.mult)

            # ---- stage weights / k / q / v (f16); v last (needed only at AV)
            wq, wk, kt = [], [], []
            for dc in range(8):
                t_ = big.tile([128, D], f16, tag="big", name="wk_sb")
                nc.sync.dma_start(t_, wkT[ts(dc, 128), :])
                wk.append(t_)
            for dc in range(8):
                t_ = big.tile([128, SKV], f16, tag="big", name="kt_sb")
                nc.sync.dma_start(t_, kT[ts(dc, 128), :])
                kt.append(t_)
            for dc in range(8):
                t_ = big.tile([128, D], f16, tag="big", name="wq_sb")
                nc.sync.dma_start(t_, wqT[ts(dc, 128), :])
                wq.append(t_)
            qt_sb = []
            for dc in range(8):
                t_ = kqp.tile([128, ROWS], f16, tag="qt", name="qt_sb")
                nc.sync.dma_start(t_, qTs[ts(dc, 128), :])
                qt_sb.append(t_)
            v_sb = vpool.tile([128, 8, D], f16, tag="v")
            for kc in range(8):
                nc.sync.dma_start(v_sb[:, kc, :], vm[ts(kc, 128), :])

            kp, qp = [], []
            e16s = {j: {} for j in range(NGRP)}   # group j -> {t: tile}
            st = {}                                # group state tiles
            at16acc = {}                           # qt -> f16 attn accumulator

            def emit_scores(j, g, E_t):
                # heads 2g, 2g+1 of group j (local h2=0,1);
                # tile index t = qt*4 + (g*2 + h2 - 4j)
                for qt in range(NQT):
                    for h2 in range(2):
                        t = qt * 4 + (g * 2 + h2 - 4 * j)
                        ps2 = pssc.tile([128, 1024], f32, tag="pssc")
                        for n in range(2):
                            nc.tensor.matmul(
                                out=ps2[:, ds(n * 512, 512)],
                                lhsT=qp[g][ds(h2 * 64, 64), ts(qt, 128)],
                                rhs=kp[g][ds(h2 * 64, 64), ds(n * 512, 512)],
                                start=True, stop=True,
                                tile_position=(h2 * 64, 0))
                        e16 = big.tile([128, SKV], f16, tag="big", name="e16")
                        nc.scalar.activation(e16, ps2, AF.Exp, scale=SCALE,
                                             accum_out=E_t[:, t:t + 1])
                        e16s[j][t] = e16

            def emit_rounds(j):
                s = st[j]
                E_t, lo, hi, thE = s["E"], s["lo"], s["hi"], s["thE"]
                T_t, N_t, m_t = s["T"], s["N"], s["m"]
                mlo, mhi, thEh = s["mlo"], s["mhi"], s["thEh"]
                lnE = small.tile([128, NTP], f32, tag="lnE")
                nc.scalar.activation(lnE, E_t, AF.Ln)
                nc.scalar.activation(lo, lnE, AF.Exp, scale=CA, bias=bias_lo)
                nc.scalar.activation(hi, lnE, AF.Exp, scale=CA, bias=bias_hi)
                nc.vector.tensor_scalar_mul(thE, E_t, TH)
                nc.vector.tensor_scalar_mul(mlo, thE, float(np.exp(-LAM * LOM)))
                nc.vector.tensor_scalar_mul(mhi, thE, float(np.exp(LAM * HIM)))
                nc.vector.tensor_scalar_mul(thEh, thE, 0.5)

                nact = N_ACT_PROBE if j < NGRP - 1 else N_ACT_PROBE_LAST
                ndve = NTP - nact
                for it in range(K_ITERS):
                    half = it < NHALF
                    wid = SKV // 2 if half else SKV
                    c_t = small.tile([128, NTP], f32, tag="c")
                    den = small.tile([128, NTP], f32, tag="den")
                    num = small.tile([128, NTP], f32, tag="num")
                    w_t = small.tile([128, NTP], f32, tag="w")
                    nc.vector.tensor_sub(den, mhi, mlo)
                    nc.vector.reciprocal(den, den)
                    nc.vector.tensor_sub(num, thE, mlo)
                    nc.vector.tensor_mul(w_t, num, den)
                    nc.vector.tensor_scalar(out=w_t, in0=w_t, scalar1=W_LO,
                                            scalar2=W_HI, op0=AL.max, op1=AL.min)
                    nc.vector.tensor_sub(c_t, hi, lo)
                    nc.vector.tensor_mul(c_t, c_t, w_t)
                    nc.vector.tensor_add(c_t, c_t, lo)
                    if half:
                        c2_t = small.tile([128, NTP], f32, tag="c2")
                        nc.vector.tensor_scalar_mul(c2_t, c_t, 2.0)
                    for t in range(NTP):
                        col = c_t[:, t:t + 1]
                        if t < ndve:
                            sT = scrT.tile([128, SKV], f16, tag="sT", name="sT")
                            nc.vector.scalar_tensor_tensor(
                                out=sT[:, ds(0, wid)], in0=e16s[j][t][:, ds(0, wid)],
                                scalar=col, in1=e16s[j][t][:, ds(0, wid)],
                                op0=AL.is_le, op1=AL.mult,
                                accum_out=m_t[:, t:t + 1])
                        else:
                            # full: R = sum relu(c-e); half: 2R via scale -2
                            sa = scrA.tile([128, SKV], f16, tag="sA", name="sa")
                            nc.scalar.activation(sa[:, ds(0, wid)],
                                                 e16s[j][t][:, ds(0, wid)], AF.Relu,
                                                 bias=(c2_t[:, t:t + 1] if half
                                                       else col),
                                                 scale=(-2.0 if half else -1.0),
                                                 accum_out=T_t[:, t:t + 1])
                            sg = scrA.tile([128, SKV], f16, tag="sA", name="sg")
                            nc.scalar.activation(sg[:, ds(0, wid)],
                                                 e16s[j][t][:, ds(0, wid)], AF.Sign,
                                                 bias=col, scale=-1.0,
                                                 accum_out=N_t[:, t:t + 1])
                    acols = ds(ndve, nact)
                    dcols = ds(0, ndve)
                    if half:
                        # m_half doubled to full-scale estimate
                        nc.vector.tensor_scalar_mul(m_t[:, dcols], m_t[:, dcols], 2.0)
                        # ACT: 2R in T_t, G' over 512; m = c*(G'+512) - 2R
                        nc.vector.tensor_scalar(
                            out=m_t[:, acols], in0=N_t[:, acols], scalar1=1.0,
                            scalar2=float(SKV // 2), op0=AL.mult, op1=AL.add)
                    else:
                        # ACT: R in T_t, G' over 1024; m = c*(G'+1024)/2 - R
                        nc.vector.tensor_scalar(
                            out=m_t[:, acols], in0=N_t[:, acols], scalar1=0.5,
                            scalar2=float(SKV // 2), op0=AL.mult, op1=AL.add)
                    nc.vector.tensor_mul(m_t[:, acols], m_t[:, acols], c_t[:, acols])
                    nc.vector.tensor_sub(m_t[:, acols], m_t[:, acols], T_t[:, acols])
                    # halved endpoint masses
                    hlo = small.tile([128, NTP], f32, tag="hlo")
                    hhi = small.tile([128, NTP], f32, tag="hhi")
                    nc.vector.scalar_tensor_tensor(
                        out=hlo, in0=mlo, scalar=0.5, in1=thEh,
                        op0=AL.mult, op1=AL.add)
                    nc.vector.scalar_tensor_tensor(
                        out=hhi, in0=mhi, scalar=0.5, in1=thEh,
                        op0=AL.mult, op1=AL.add)
                    # branch update
                    sel = small.tile([128, NTP], mybir.dt.uint8, tag="sel")
                    nc.vector.tensor_tensor(out=sel, in0=m_t, in1=thE, op=AL.is_lt)
                    nc.vector.copy_predicated(lo, sel, c_t)
                    nc.vector.copy_predicated(mlo, sel, m_t)
                    nc.vector.copy_predicated(mhi, sel, hhi)
                    nc.vector.tensor_tensor(out=sel, in0=m_t, in1=thE, op=AL.is_ge)
                    nc.vector.copy_predicated(hi, sel, c_t)
                    nc.vector.copy_predicated(mhi, sel, m_t)
                    nc.vector.copy_predicated(mlo, sel, hlo)

            def emit_av(qt):
                at16 = at16acc[qt]
                nc.sync.dma_start(attn_o[ts(qt, 128), :], at16)
                aTs = []
                for kc in range(8):
                    pst = ps512.tile([128, 128], f16, tag="ps512", name="pst")
                    nc.tensor.transpose(pst, at16[:, ts(kc, 128)], ident)
                    aT = aTp.tile([128, 128], f16, tag="aT")
                    nc.vector.tensor_copy(aT, pst)
                    aTs.append(aT)
                for half in range(2):
                    po = ps512.tile([128, 512], f32, tag="ps512")
                    for kc in range(8):
                        nc.tensor.matmul(
                            out=po, lhsT=aTs[kc],
                            rhs=v_sb[:, kc, ds(half * 512, 512)],
                            start=(kc == 0), stop=(kc == 7))
                    osb = osbp.tile([128, 512], f16, tag="osb")
                    nc.scalar.copy(osb, po)
                    nc.sync.dma_start(out_o[ts(qt, 128), ds(half * 512, 512)], osb)

            def emit_partial(j):
                # masks (with S accumulation) for the whole group, then r2,
                # then diag-scale + PE-accumulate and fold into the f16
                # accumulator (pool does the adds)
                s = st[j]
                lo, E_t, S_t, r2_t = s["lo"], s["E"], s["S"], s["r2"]
                mks = {}
                for qt in range(NQT):
                    for hh in range(4):
                        col = qt * 4 + hh
                        mk = maskp.tile([128, SKV], f16, tag="mk", name="mk")
                        nc.vector.scalar_tensor_tensor(
                            out=mk, in0=e16s[j][col], scalar=lo[:, col:col + 1],
                            in1=e16s[j][col], op0=AL.is_gt, op1=AL.mult,
                            accum_out=S_t[:, col:col + 1])
                        mks[col] = mk
                nc.vector.scalar_tensor_tensor(
                    out=S_t, in0=E_t, scalar=EPS, in1=S_t,
                    op0=AL.mult, op1=AL.add)
                nc.vector.reciprocal(r2_t, S_t)
                for qt in range(NQT):
                    pa = [psat.tile([128, 512], f32, tag="psat", name="pa")
                          for _ in range(2)]
                    last = j > 0
                    for hh in range(4):
                        col = qt * 4 + hh
                        mk = mks[col]
                        dg = dgp.tile([128, 128], f16, tag="dg", name="dg")
                        nc.vector.tensor_scalar(
                            out=dg, in0=ident, scalar1=r2_t[:, col:col + 1],
                            scalar2=None, op0=AL.mult)
                        for half in range(2):
                            nc.tensor.matmul(
                                out=pa[half], lhsT=dg,
                                rhs=mk[:, ds(half * 512, 512)],
                                start=(hh == 0),
                                stop=(hh == 3 and not last))
                    if last:
                        # fold the previous groups' accumulated attn back in:
                        # psum += H * at16acc; the final copy divides by H
                        for half in range(2):
                            nc.tensor.matmul(
                                out=pa[half], lhsT=hident,
                                rhs=at16acc[qt][:, ds(half * 512, 512)],
                                start=False, stop=True)
                    acc = atacc.tile([128, SKV], f16, tag="acc", name="acc")
                    for half in range(2):
                        nc.scalar.activation(acc[:, ds(half * 512, 512)],
                                             pa[half], AF.Copy, scale=1.0 / H)
                    at16acc[qt] = acc
                    if j == NGRP - 1:
                        emit_av(qt)

            # ---- projections interleaved with scores/rounds of ready groups
            for g in range(8):
                kp_g = kqp.tile([128, SKV], f16, tag="kp", name="kp_g", bufs=3)
                for half in range(2):
                    ps = ps512.tile([128, 512], f32, tag="ps512")
                    for dc in range(8):
                        nc.tensor.matmul(out=ps, lhsT=wk[dc][:, ts(g, 128)],
                                         rhs=kt[dc][:, ds(half * 512, 512)],
                                         start=(dc == 0), stop=(dc == 7))
                    nc.vector.tensor_copy(kp_g[:, ds(half * 512, 512)], ps)
                kp.append(kp_g)
                qp_g = kqp.tile([128, ROWS], f16, tag="qp", name="qp_g", bufs=3)
                ps = ps512.tile([128, 512], f32, tag="ps512")
                for dc in range(8):
                    nc.tensor.matmul(out=ps, lhsT=wq[dc][:, ts(g, 128)],
                                     rhs=qt_sb[dc],
                                     start=(dc == 0), stop=(dc == 7))
                nc.vector.tensor_copy(qp_g, ps)
                qp.append(qp_g)
                if g % 2 == 1:
                    j = g // 2
                    st[j] = {nm: small.tile([128, NTP], f32, tag=nm, name=nm)
                             for nm in ("E", "lo", "hi", "thE", "T", "N", "m",
                                        "mlo", "mhi", "thEh", "S", "r2")}
                    emit_scores(j, g - 1, st[j]["E"])
                    emit_scores(j, g, st[j]["E"])
                    emit_rounds(j)
                    emit_partial(j)

    nc.compile()
    return nc


def _get_module():
    if "nc" not in _CACHE:
        _CACHE["nc"] = _build_module()
    return _CACHE["nc"]


def kernel(q, k, v, Wq, Wk, k_mask=None):
    from concourse.bass_utils import run_bass_kernel_spmd

    nc = _get_module()
    f16 = np.float16
    qT = np.ascontiguousarray(q.transpose(0, 2, 1)).astype(f16)   # (B, D, SQ)
    kTf = np.ascontiguousarray(k.transpose(0, 2, 1)).astype(f16)  # (B, D, SKV)
    v16 = np.ascontiguousarray(v).astype(f16)
    wqT = np.ascontiguousarray(Wq.T).astype(f16)
    wkT = np.ascontiguousarray(Wk.T).astype(f16)
    in_maps = []
    for c in range(NCORES):
        b, r = c // 2, c % 2
        in_maps.append({
            "qTs": np.ascontiguousarray(qT[b][:, r * ROWS:(r + 1) * ROWS]),
            "kT": kTf[b],
            "vm": v16[b],
            "wqT": wqT,
            "wkT": wkT,
        })
    res = run_bass_kernel_spmd(nc, in_maps, core_ids=list(range(NCORES)))
    _CACHE["last_res"] = res
    attn = np.empty((B, SQ, SKV), np.float32)
    out = np.empty((B, SQ, D), np.float32)
    for c in range(NCORES):
        b, r = c // 2, c % 2
        attn[b, r * ROWS:(r + 1) * ROWS, :] = res.results[c]["attn_p"]
        out[b, r * ROWS:(r + 1) * ROWS, :] = res.results[c]["out_p"]
    return out, attn


# revision 23
# speedup vs baseline: 1.0290x; 1.0085x over previous
"""Trainium2 Bass kernel for ConfigurableMultiHeadAttention with
cum-thresholded (top-p style) softmax.

Sharding: data-parallel over (batch x query-row-half) -- core c handles
batch c//2, query rows (c%2)*512 .. +512, and ALL 16 heads for those
rows.  The head-mean of the attention matrix is core-local (no
collective, no host-side reduction); each core writes its own 512-row
slice of attn and of out = attn @ v.  Host just concatenates.

All matmuls run in fp16 (f32 PSUM accumulate): q/k/v and the weights
are cast to f16 on the host.

Cum-thresholded softmax without sort/cumsum: find the per-row cutoff
value c* where the ascending cumulative mass crosses 0.1*E, keep
e > lo, renormalize by the actual kept mass.  Root-finding is Illinois
false position (K=3), warm-started from a logE regression; probes are
fused compare-mult-accumulate ops on DVE plus Relu/Sign pairs on ACT.

Probe groups are HEAD-pair groups (heads 4j..4j+3 across all four
q-tiles): a group's scores only need two projection column blocks, so
its bisection starts while later projections still run, and different
groups' round chains interleave on the engines (no global round
barrier).  The kept mass S comes from tracking m at the accepted lower
bound during the rounds, so finalize masks need no accumulation -- the
per-head mask tiles stream straight into identity-diag matmuls on the
otherwise idle PE, which performs the 16-head mean in PSUM.
"""

import numpy as np

B, SQ, SKV, D, H, DH = 4, 1024, 1024, 1024, 16, 64
NCORES = 8
ROWS = 512          # q rows per core
NQT = ROWS // 128   # q-tiles per core (4)
NGRP = 4            # head-pair groups (heads 4j..4j+3)
NTP = 16            # probe tiles per group (4 qtiles x 4 heads)
K_ITERS = 3
CA, CB = 0.73950811, -5.77096104
LOM, HIM = 0.1418, 0.1865  # search window margins around predictor
TH, EPS, SCALE = 0.1, 1e-7, 0.125
LAM = 1.75                # model slope for initial endpoint masses
W_LO, W_HI = 0.04, 0.96   # false-position weight clip
N_ACT_PER_GRP = (6, 6, 5, 7)  # probe tiles per group on ACT (Relu+Sign pair)
NHALF = 1                 # early rounds probe only the first 512 kv columns

_CACHE = {}


def _build_module():
    import concourse.bacc as bacc
    import concourse.mybir as mybir
    from concourse.tile import TileContext
    from concourse.bass import ds, ts
    from concourse.masks import make_identity

    f32, f16 = mybir.dt.float32, mybir.dt.float16
    AL = mybir.AluOpType
    AF = mybir.ActivationFunctionType

    nc = bacc.Bacc("TRN2", target_bir_lowering=False, debug=False,
                   enable_asserts=False, num_devices=NCORES)
    qTs = nc.dram_tensor("qTs", (D, ROWS), f16, kind="ExternalInput").ap()
    kT = nc.dram_tensor("kT", (D, SKV), f16, kind="ExternalInput").ap()
    vm = nc.dram_tensor("vm", (SKV, D), f16, kind="ExternalInput").ap()
    wqT = nc.dram_tensor("wqT", (D, D), f16, kind="ExternalInput").ap()
    wkT = nc.dram_tensor("wkT", (D, D), f16, kind="ExternalInput").ap()
    attn_o = nc.dram_tensor("attn_p", (ROWS, SKV), f16, kind="ExternalOutput").ap()
    out_o = nc.dram_tensor("out_p", (ROWS, D), f16, kind="ExternalOutput").ap()

    from contextlib import ExitStack
    with TileContext(nc) as tc:
        with ExitStack() as stk:
            # [128,1024] f16 slots shared by weight/k staging and e16
            big = stk.enter_context(tc.tile_pool(name="big", bufs=52))
            kqp = stk.enter_context(tc.tile_pool(name="kqp", bufs=8))
            vpool = stk.enter_context(tc.tile_pool(name="vp", bufs=1))
            maskp = stk.enter_context(tc.tile_pool(name="maskp", bufs=18))
            dgp = stk.enter_context(tc.tile_pool(name="dgp", bufs=6))
            atacc = stk.enter_context(tc.tile_pool(name="atacc", bufs=6))
            scrT = stk.enter_context(tc.tile_pool(name="scrT", bufs=2))
            scrA = stk.enter_context(tc.tile_pool(name="scrA", bufs=2))
            small = stk.enter_context(tc.tile_pool(name="small", bufs=5))
            aTp = stk.enter_context(tc.tile_pool(name="aTp", bufs=8))
            osbp = stk.enter_context(tc.tile_pool(name="osb", bufs=3))
            wcons = stk.enter_context(tc.tile_pool(name="wcons", bufs=1))
            pssc = stk.enter_context(tc.tile_pool(name="pssc", bufs=2, space="PSUM"))
            ps512 = stk.enter_context(tc.tile_pool(name="ps512", bufs=2, space="PSUM"))
            psat = stk.enter_context(tc.tile_pool(name="psat", bufs=2, space="PSUM"))

            bias_lo = wcons.tile([128, 1], f32, tag="blo")
            bias_hi = wcons.tile([128, 1], f32, tag="bhi")
            nc.vector.memset(bias_lo, CB - LOM)
            nc.vector.memset(bias_hi, CB + HIM)
            ident = wcons.tile([128, 128], f16, tag="ident")
            make_identity(nc, ident)
            hident = wcons.tile([128, 128], f16, tag="hident")
            nc.vector.tensor_scalar(out=hident, in0=ident, scalar1=float(H),
                                    scalar2=None, op0=AL.mult)

            # ---- stage weights / k / q / v (f16); v last (needed only at AV)
            wq, wk, kt = [], [], []
            for dc in range(8):
                t_ = big.tile([128, D], f16, tag="big", name="wk_sb")
                nc.sync.dma_start(t_, wkT[ts(dc, 128), :])
                wk.append(t_)
            for dc in range(8):
                t_ = big.tile([128, SKV], f16, tag="big", name="kt_sb")
                nc.sync.dma_start(t_, kT[ts(dc, 128), :])
                kt.append(t_)
            for dc in range(8):
                t_ = big.tile([128, D], f16, tag="big", name="wq_sb")
                nc.sync.dma_start(t_, wqT[ts(dc, 128), :])
                wq.append(t_)
            qt_sb = []
            for dc in range(8):
                t_ = kqp.tile([128, ROWS], f16, tag="qt", name="qt_sb")
                nc.sync.dma_start(t_, qTs[ts(dc, 128), :])
                qt_sb.append(t_)
            v_sb = vpool.tile([128, 8, D], f16, tag="v")
            for kc in range(8):
                nc.sync.dma_start(v_sb[:, kc, :], vm[ts(kc, 128), :])

            kp, qp = [], []
            e16s = {j: {} for j in range(NGRP)}   # group j -> {t: tile}
            st = {}                                # group state tiles
            at16acc = {}                           # qt -> f16 attn accumulator

            def emit_scores(j, g, E_t):
                # heads 2g, 2g+1 of group j (local h2=0,1);
                # tile index t = qt*4 + (g*2 + h2 - 4j)
                for qt in range(NQT):
                    for h2 in range(2):
                        t = qt * 4 + (g * 2 + h2 - 4 * j)
                        ps2 = pssc.tile([128, 1024], f32, tag="pssc")
                        for n in range(2):
                            nc.tensor.matmul(
                                out=ps2[:, ds(n * 512, 512)],
                                lhsT=qp[g][ds(h2 * 64, 64), ts(qt, 128)],
                                rhs=kp[g][ds(h2 * 64, 64), ds(n * 512, 512)],
                                start=True, stop=True,
                                tile_position=(h2 * 64, 0))
                        e16 = big.tile([128, SKV], f16, tag="big", name="e16")
                        nc.scalar.activation(e16, ps2, AF.Exp, scale=SCALE,
                                             accum_out=E_t[:, t:t + 1])
                        e16s[j][t] = e16

            def emit_rounds(j):
                s = st[j]
                E_t, lo, hi, thE = s["E"], s["lo"], s["hi"], s["thE"]
                T_t, N_t, m_t = s["T"], s["N"], s["m"]
                mlo, mhi, thEh = s["mlo"], s["mhi"], s["thEh"]
                lnE = small.tile([128, NTP], f32, tag="lnE")
                nc.scalar.activation(lnE, E_t, AF.Ln)
                nc.scalar.activation(lo, lnE, AF.Exp, scale=CA, bias=bias_lo)
                nc.scalar.activation(hi, lnE, AF.Exp, scale=CA, bias=bias_hi)
                nc.vector.tensor_scalar_mul(thE, E_t, TH)
                nc.vector.tensor_scalar_mul(mlo, thE, float(np.exp(-LAM * LOM)))
                nc.vector.tensor_scalar_mul(mhi, thE, float(np.exp(LAM * HIM)))
                nc.vector.tensor_scalar_mul(thEh, thE, 0.5)

                nact = N_ACT_PER_GRP[j]
                ndve = NTP - nact
                for it in range(K_ITERS):
                    half = it < NHALF
                    wid = SKV // 2 if half else SKV
                    c_t = small.tile([128, NTP], f32, tag="c")
                    den = small.tile([128, NTP], f32, tag="den")
                    num = small.tile([128, NTP], f32, tag="num")
                    w_t = small.tile([128, NTP], f32, tag="w")
                    nc.vector.tensor_sub(den, mhi, mlo)
                    nc.vector.reciprocal(den, den)
                    nc.vector.tensor_sub(num, thE, mlo)
                    nc.vector.tensor_mul(w_t, num, den)
                    nc.vector.tensor_scalar(out=w_t, in0=w_t, scalar1=W_LO,
                                            scalar2=W_HI, op0=AL.max, op1=AL.min)
                    nc.vector.tensor_sub(c_t, hi, lo)
                    nc.vector.tensor_mul(c_t, c_t, w_t)
                    nc.vector.tensor_add(c_t, c_t, lo)
                    if half:
                        c2_t = small.tile([128, NTP], f32, tag="c2")
                        nc.vector.tensor_scalar_mul(c2_t, c_t, 2.0)
                    for t in range(NTP):
                        col = c_t[:, t:t + 1]
                        if t < ndve:
                            sT = scrT.tile([128, SKV], f16, tag="sT", name="sT")
                            nc.vector.scalar_tensor_tensor(
                                out=sT[:, ds(0, wid)], in0=e16s[j][t][:, ds(0, wid)],
                                scalar=col, in1=e16s[j][t][:, ds(0, wid)],
                                op0=AL.is_le, op1=AL.mult,
                                accum_out=m_t[:, t:t + 1])
                        else:
                            # full: R = sum relu(c-e); half: 2R via scale -2
                            sa = scrA.tile([128, SKV], f16, tag="sA", name="sa")
                            nc.scalar.activation(sa[:, ds(0, wid)],
                                                 e16s[j][t][:, ds(0, wid)], AF.Relu,
                                                 bias=(c2_t[:, t:t + 1] if half
                                                       else col),
                                                 scale=(-2.0 if half else -1.0),
                                                 accum_out=T_t[:, t:t + 1])
                            sg = scrA.tile([128, SKV], f16, tag="sA", name="sg")
                            nc.scalar.activation(sg[:, ds(0, wid)],
                                                 e16s[j][t][:, ds(0, wid)], AF.Sign,
                                                 bias=col, scale=-1.0,
                                                 accum_out=N_t[:, t:t + 1])
                    acols = ds(ndve, nact)
                    dcols = ds(0, ndve)
                    if half:
                        # m_half doubled to full-scale estimate
                        nc.vector.tensor_scalar_mul(m_t[:, dcols], m_t[:, dcols], 2.0)
                        # ACT: 2R in T_t, G' over 512; m = c*(G'+512) - 2R
                        nc.vector.tensor_scalar(
                            out=m_t[:, acols], in0=N_t[:, acols], scalar1=1.0,
                            scalar2=float(SKV // 2), op0=AL.mult, op1=AL.add)
                    else:
                        # ACT: R in T_t, G' over 1024; m = c*(G'+1024)/2 - R
                        nc.vector.tensor_scalar(
                            out=m_t[:, acols], in0=N_t[:, acols], scalar1=0.5,
                            scalar2=float(SKV // 2), op0=AL.mult, op1=AL.add)
                    nc.vector.tensor_mul(m_t[:, acols], m_t[:, acols], c_t[:, acols])
                    nc.vector.tensor_sub(m_t[:, acols], m_t[:, acols], T_t[:, acols])
                    # halved endpoint masses
                    hlo = small.tile([128, NTP], f32, tag="hlo")
                    hhi = small.tile([128, NTP], f32, tag="hhi")
                    nc.vector.scalar_tensor_tensor(
                        out=hlo, in0=mlo, scalar=0.5, in1=thEh,
                        op0=AL.mult, op1=AL.add)
                    nc.vector.scalar_tensor_tensor(
                        out=hhi, in0=mhi, scalar=0.5, in1=thEh,
                        op0=AL.mult, op1=AL.add)
                    # branch update
                    sel = small.tile([128, NTP], mybir.dt.uint8, tag="sel")
                    nc.vector.tensor_tensor(out=sel, in0=m_t, in1=thE, op=AL.is_lt)
                    nc.vector.copy_predicated(lo, sel, c_t)
                    nc.vector.copy_predicated(mlo, sel, m_t)
                    nc.vector.copy_predicated(mhi, sel, hhi)
                    nc.vector.tensor_tensor(out=sel, in0=m_t, in1=thE, op=AL.is_ge)
                    nc.vector.copy_predicated(hi, sel, c_t)
                    nc.vector.copy_predicated(mhi, sel, m_t)
                    nc.vector.copy_predicated(mlo, sel, hlo)

            def emit_av(qt):
                at16 = at16acc[qt]
                nc.sync.dma_start(attn_o[ts(qt, 128), :], at16)
                aTs = []
                for kc in range(8):
                    pst = ps512.tile([128, 128], f16, tag="ps512", name="pst")
                    nc.tensor.transpose(pst, at16[:, ts(kc, 128)], ident)
                    aT = aTp.tile([128, 128], f16, tag="aT")
                    nc.vector.tensor_copy(aT, pst)
                    aTs.append(aT)
                for half in range(2):
                    po = ps512.tile([128, 512], f32, tag="ps512")
                    for kc in range(8):
                        nc.tensor.matmul(
                            out=po, lhsT=aTs[kc],
                            rhs=v_sb[:, kc, ds(half * 512, 512)],
                            start=(kc == 0), stop=(kc == 7))
                    osb = osbp.tile([128, 512], f16, tag="osb")
                    nc.scalar.copy(osb, po)
                    nc.sync.dma_start(out_o[ts(qt, 128), ds(half * 512, 512)], osb)

            def emit_partial(j):
                # masks (with S accumulation) for the whole group, then r2,
                # then diag-scale + PE-accumulate and fold into the f16
                # accumulator (pool does the adds)
                s = st[j]
                lo, E_t, S_t, r2_t = s["lo"], s["E"], s["S"], s["r2"]
                mks = {}
                for qt in range(NQT):
                    for hh in range(4):
                        col = qt * 4 + hh
                        mk = maskp.tile([128, SKV], f16, tag="mk", name="mk")
                        nc.vector.scalar_tensor_tensor(
                            out=mk, in0=e16s[j][col], scalar=lo[:, col:col + 1],
                            in1=e16s[j][col], op0=AL.is_gt, op1=AL.mult,
                            accum_out=S_t[:, col:col + 1])
                        mks[col] = mk
                nc.vector.scalar_tensor_tensor(
                    out=S_t, in0=E_t, scalar=EPS, in1=S_t,
                    op0=AL.mult, op1=AL.add)
                nc.vector.reciprocal(r2_t, S_t)
                for qt in range(NQT):
                    pa = [psat.tile([128, 512], f32, tag="psat", name="pa")
                          for _ in range(2)]
                    last = j > 0
                    for hh in range(4):
                        col = qt * 4 + hh
                        mk = mks[col]
                        dg = dgp.tile([128, 128], f16, tag="dg", name="dg")
                        nc.vector.tensor_scalar(
                            out=dg, in0=ident, scalar1=r2_t[:, col:col + 1],
                            scalar2=None, op0=AL.mult)
                        for half in range(2):
                            nc.tensor.matmul(
                                out=pa[half], lhsT=dg,
                                rhs=mk[:, ds(half * 512, 512)],
                                start=(hh == 0),
                                stop=(hh == 3 and not last))
                    if last:
                        # fold the previous groups' accumulated attn back in:
                        # psum += H * at16acc; the final copy divides by H
                        for half in range(2):
                            nc.tensor.matmul(
                                out=pa[half], lhsT=hident,
                                rhs=at16acc[qt][:, ds(half * 512, 512)],
                                start=False, stop=True)
                    acc = atacc.tile([128, SKV], f16, tag="acc", name="acc")
                    for half in range(2):
                        if j == NGRP - 1:
                            nc.vector.tensor_scalar(
                                out=acc[:, ds(half * 512, 512)], in0=pa[half],
                                scalar1=1.0 / H, scalar2=None, op0=AL.mult)
                        else:
                            nc.scalar.activation(acc[:, ds(half * 512, 512)],
                                                 pa[half], AF.Copy, scale=1.0 / H)
                    at16acc[qt] = acc
                    if j == NGRP - 1:
                        emit_av(qt)

            # ---- projections interleaved with scores/rounds of ready groups
            for g in range(8):
                kp_g = kqp.tile([128, SKV], f16, tag="kp", name="kp_g", bufs=3)
                for half in range(2):
                    ps = ps512.tile([128, 512], f32, tag="ps512")
                    for dc in range(8):
                        nc.tensor.matmul(out=ps, lhsT=wk[dc][:, ts(g, 128)],
                                         rhs=kt[dc][:, ds(half * 512, 512)],
                                         start=(dc == 0), stop=(dc == 7))
                    nc.vector.tensor_copy(kp_g[:, ds(half * 512, 512)], ps)
                kp.append(kp_g)
                qp_g = kqp.tile([128, ROWS], f16, tag="qp", name="qp_g", bufs=3)
                ps = ps512.tile([128, 512], f32, tag="ps512")
                for dc in range(8):
                    nc.tensor.matmul(out=ps, lhsT=wq[dc][:, ts(g, 128)],
                                     rhs=qt_sb[dc],
                                     start=(dc == 0), stop=(dc == 7))
                nc.vector.tensor_copy(qp_g, ps)
                qp.append(qp_g)
                if g % 2 == 1:
                    j = g // 2
                    st[j] = {nm: small.tile([128, NTP], f32, tag=nm, name=nm)
                             for nm in ("E", "lo", "hi", "thE", "T", "N", "m",
                                        "mlo", "mhi", "thEh", "S", "r2")}
                    emit_scores(j, g - 1, st[j]["E"])
                    emit_scores(j, g, st[j]["E"])
                    emit_rounds(j)
                    emit_partial(j)

    nc.compile()
    return nc


def _get_module():
    if "nc" not in _CACHE:
        _CACHE["nc"] = _build_module()
    return _CACHE["nc"]


def kernel(q, k, v, Wq, Wk, k_mask=None):
    from concourse.bass_utils import run_bass_kernel_spmd

    nc = _get_module()
    f16 = np.float16
    qT = np.ascontiguousarray(q.transpose(0, 2, 1)).astype(f16)   # (B, D, SQ)
    kTf = np.ascontiguousarray(k.transpose(0, 2, 1)).astype(f16)  # (B, D, SKV)
    v16 = np.ascontiguousarray(v).astype(f16)
    wqT = np.ascontiguousarray(Wq.T).astype(f16)
    wkT = np.ascontiguousarray(Wk.T).astype(f16)
    in_maps = []
    for c in range(NCORES):
        b, r = c // 2, c % 2
        in_maps.append({
            "qTs": np.ascontiguousarray(qT[b][:, r * ROWS:(r + 1) * ROWS]),
            "kT": kTf[b],
            "vm": v16[b],
            "wqT": wqT,
            "wkT": wkT,
        })
    res = run_bass_kernel_spmd(nc, in_maps, core_ids=list(range(NCORES)))
    _CACHE["last_res"] = res
    attn = np.empty((B, SQ, SKV), np.float32)
    out = np.empty((B, SQ, D), np.float32)
    for c in range(NCORES):
        b, r = c // 2, c % 2
        attn[b, r * ROWS:(r + 1) * ROWS, :] = res.results[c]["attn_p"]
        out[b, r * ROWS:(r + 1) * ROWS, :] = res.results[c]["out_p"]
    return out, attn
